# revision 6
# baseline (speedup 1.0000x reference)
"""Trainium2 Bass kernel for LLN+diag attention (v3).

out = 0.5 * (lln_linear_attention(q,k,v) + block_diag_attention(q,k,v))

Shapes: q,k,v [4,16,4096,64] fp32.  8 NeuronCores, 8 heads per core.

Host prep (sharding/layout only; global std scalars are cross-device):
  qt2 [4,128,4096] bf16  pair-packed (alpha*q)^T  (exp -> lin Q; scores)
  kt2 [4,128,4096] bf16  pair-packed (k/(8 alpha))^T  (scores: qt*kt = q*k/8)
  ke  [8,128,32,64] bf16  exp(beta*k) n-major, PRE-EXPONENTIATED on host
  va  [8,128,32,65] bf16  v n-major + ones column of value 2.0
  out [8,128,32,64] bf16  device result; host transposes + upcasts to fp32

All DMA transfers are contiguous per-partition lines (no gather).  qt2/kt2
are chunked into 4 column tiles so score/linear matmuls start as soon as
their chunk lands (cuts the pipeline fill).  The value-2.0 ones column
doubles both paths' denominators, so adding the two normalized halves
yields the required 0.5*(lin+diag).

PSUM note: a matmul output chunk must not cross a 2KB PSUM bank line
(probed: crossing chunks corrupt), so group tiles are <= [128,7,65] = 1820B.
Group sizes [7,7,2 | 7,7,2] put a boundary at n-tile 16 so each half-head
output add/DMA fires as soon as its half is done.
"""

import math
import os
import sys

for _p in ("/opt/trn_rl_repo", "/opt/pypackages"):
    if os.path.isdir(_p) and _p not in sys.path:
        sys.path.insert(0, _p)

import numpy as np
import ml_dtypes

B, H, N, D = 4, 16, 4096, 64
N_CORES = 8
HPC = (B * H) // N_CORES          # heads per core = 8
NT = N // 128                     # 128-row n-tiles per head = 32
GS = [7, 7, 2, 7, 7, 2]           # group sizes; boundary at 16 for half adds
GOFF = [0, 7, 14, 16, 23, 30]
NCHUNK = 4                        # qt2/kt2/qte column chunks of 1024
CW = N // NCHUNK
A_CONST = 0.14855178144710912
B_CONST = -0.35487039130661086

_BF16 = ml_dtypes.bfloat16

_cache = {}


def _build():
    import concourse.bass as bass
    import concourse.bacc as bacc
    import concourse.mybir as mybir
    from concourse.tile import TileContext

    dt = mybir.dt
    F32, BF = dt.float32, dt.bfloat16
    Exp = mybir.ActivationFunctionType.Exp
    Copy = mybir.ActivationFunctionType.Copy
    MUL = mybir.AluOpType.mult
    ADD = mybir.AluOpType.add

    nc = bacc.Bacc()
    qt2_d = nc.dram_tensor("qt2", [HPC // 2, 128, N], BF, kind="ExternalInput")
    kt2_d = nc.dram_tensor("kt2", [HPC // 2, 128, N], BF, kind="ExternalInput")
    ke_d = nc.dram_tensor("ke", [HPC, 128, NT, D], BF, kind="ExternalInput")
    va_d = nc.dram_tensor("va", [HPC, 128, NT, D + 1], BF, kind="ExternalInput")
    out_d = nc.dram_tensor("out", [HPC, 128, NT, D], BF, kind="ExternalOutput")

    with TileContext(nc) as tc:
        from contextlib import ExitStack

        with ExitStack() as ctx:
            pair_p = ctx.enter_context(tc.tile_pool(name="pair", bufs=2))
            head_p = ctx.enter_context(tc.tile_pool(name="head", bufs=2))
            kva_p = ctx.enter_context(tc.tile_pool(name="kva", bufs=2))
            at_p = ctx.enter_context(tc.tile_pool(name="attn", bufs=3))
            r_p = ctx.enter_context(tc.tile_pool(name="recip", bufs=4))
            t_p = ctx.enter_context(tc.tile_pool(name="tprod", bufs=2))
            o_p = ctx.enter_context(tc.tile_pool(name="outp", bufs=2))
            kv_ps_p = ctx.enter_context(tc.tile_pool(name="kvps", bufs=1, space="PSUM"))
            sc_ps_p = ctx.enter_context(tc.tile_pool(name="scps", bufs=2, space="PSUM"))
            da_ps_p = ctx.enter_context(tc.tile_pool(name="daps", bufs=3, space="PSUM"))
            li_ps_p = ctx.enter_context(tc.tile_pool(name="lips", bufs=2, space="PSUM"))

            for p in range(HPC // 2):  # head pairs; head 2p on parts 0:64, 2p+1 on 64:128
                # ---- DMAs: head0's kv operands first so PE starts ASAP ----
                ke0 = head_p.tile([128, NT, D], BF, tag="ke0")
                nc.sync.dma_start(ke0[:], ke_d[2 * p])
                va0 = head_p.tile([128, NT, D + 1], BF, tag="va0")
                nc.sync.dma_start(va0[:], va_d[2 * p])

                kt2c, qt2c, qtec = [], [], []
                for c in range(NCHUNK):
                    kt = pair_p.tile([128, CW], BF, tag=f"kt2c{c}", name=f"kt2c{c}")
                    qt = pair_p.tile([128, CW], BF, tag=f"qt2c{c}", name=f"qt2c{c}")
                    kt2c.append(kt)
                    qt2c.append(qt)
                    qtec.append(
                        pair_p.tile([128, CW], BF, tag=f"qtec{c}", name=f"qtec{c}")
                    )
                if p == 0:
                    # chunked loads: score/linear work starts on chunk 0
                    for c in range(NCHUNK):
                        if c == 1:
                            ke1 = head_p.tile([128, NT, D], BF, tag="ke1")
                            nc.sync.dma_start(ke1[:], ke_d[2 * p + 1])
                            va1 = head_p.tile([128, NT, D + 1], BF, tag="va1")
                            nc.sync.dma_start(va1[:], va_d[2 * p + 1])
                        nc.sync.dma_start(
                            kt2c[c][:], kt2_d[p, :, c * CW : (c + 1) * CW]
                        )
                        nc.sync.dma_start(
                            qt2c[c][:], qt2_d[p, :, c * CW : (c + 1) * CW]
                        )
                        nc.scalar.activation(qtec[c][:], qt2c[c][:], Exp)
                else:
                    for c in range(NCHUNK):
                        nc.sync.dma_start(
                            kt2c[c][:], kt2_d[p, :, c * CW : (c + 1) * CW]
                        )
                    for c in range(NCHUNK):
                        nc.sync.dma_start(
                            qt2c[c][:], qt2_d[p, :, c * CW : (c + 1) * CW]
                        )
                    ke1 = head_p.tile([128, NT, D], BF, tag="ke1")
                    nc.sync.dma_start(ke1[:], ke_d[2 * p + 1])
                    va1 = head_p.tile([128, NT, D + 1], BF, tag="va1")
                    nc.sync.dma_start(va1[:], va_d[2 * p + 1])
                    for c in range(NCHUNK):
                        nc.scalar.activation(qtec[c][:], qt2c[c][:], Exp)

                kes, vas = [ke0, ke1], [va0, va1]
                t1s, t2s, outs = [], [], []
                for hh in range(2):
                    t1s.append(
                        [
                            t_p.tile([128, 16, D], BF, tag=f"t1h{hh}x{x}", name=f"t1h{hh}x{x}")
                            for x in range(2)
                        ]
                    )
                    t2s.append(
                        [
                            t_p.tile([128, 16, D], BF, tag=f"t2h{hh}x{x}", name=f"t2h{hh}x{x}")
                            for x in range(2)
                        ]
                    )
                    outs.append(
                        [
                            o_p.tile([128, 16, D], BF, tag=f"oh{hh}x{x}", name=f"oh{hh}x{x}")
                            for x in range(2)
                        ]
                    )

                kva = kva_p.tile([128, D + 1], BF, tag="kva")

                kv_ps = kv_ps_p.tile([128, D + 1], F32, tag="kv", name="kv")

                def kv_chain(hh):
                    ke, va = kes[hh], vas[hh]
                    for a in range(NT):
                        nc.tensor.matmul(
                            kv_ps[64 * hh : 64 * hh + 64, :],
                            lhsT=ke[:, a, :],
                            rhs=va[:, a, :],
                            start=(a == 0),
                            stop=(a == NT - 1),
                            tile_position=(0, 64 * hh),
                        )
                    nc.scalar.activation(
                        kva[64 * hh : 64 * hh + 64, :],
                        kv_ps[64 * hh : 64 * hh + 64, :],
                        Copy,
                    )

                def group(g, hh):
                    goff, gn = GOFF[g], GS[g]
                    half_ix = 0 if g < 3 else 1
                    toff = goff - 16 * half_ix
                    hp = 64 * hh
                    va = vas[hh]
                    # -- block-diag scores^T: 2 blocks per a-tile --
                    sc_ps = sc_ps_p.tile([128, 7, D], F32, tag="sc", name="sc")
                    for j in range(2 * gn):
                        i = j >> 1
                        half = j & 1
                        b = 2 * (goff + i) + half
                        c, bc = b // 16, b % 16
                        nc.tensor.matmul(
                            sc_ps[64 * half : 64 * half + 64, i, :],
                            lhsT=kt2c[c][hp : hp + 64, 64 * bc : 64 * bc + 64],
                            rhs=qt2c[c][hp : hp + 64, 64 * bc : 64 * bc + 64],
                            start=True,
                            stop=True,
                            tile_position=(hp, 64 * half),
                        )
                    at_sb = at_p.tile([128, 7, D], BF, tag="at", name="at")
                    nc.scalar.activation(at_sb[:, 0:gn, :], sc_ps[:, 0:gn, :], Exp)
                    # -- linear path numerators + 2S column (hides exp latency) --
                    li_ps = li_ps_p.tile([128, 7, D + 1], F32, tag="li", name="li")
                    for i in range(gn):
                        a = goff + i
                        c, ac = a // 8, a % 8
                        nc.tensor.matmul(
                            li_ps[:, i, :],
                            lhsT=qtec[c][hp : hp + 64, 128 * ac : 128 * ac + 128],
                            rhs=kva[hp : hp + 64, :],
                            start=True,
                            stop=True,
                            tile_position=(hp, 0),
                        )
                    # -- block-diag numerators + 2S column --
                    da_ps = da_ps_p.tile([128, 7, D + 1], F32, tag="da", name="da")
                    for j in range(2 * gn):
                        i = j >> 1
                        half = j & 1
                        nc.tensor.matmul(
                            da_ps[64 * half : 64 * half + 64, i, :],
                            lhsT=at_sb[64 * half : 64 * half + 64, i, :],
                            rhs=va[64 * half : 64 * half + 64, goff + i, :],
                            start=True,
                            stop=True,
                            tile_position=(64 * half, 64 * half),
                        )
                    # -- normalize both paths (DVE reads PSUM) --
                    rl = r_p.tile([128, 7], F32, tag="rl", name="rl")
                    nc.vector.reciprocal(rl[:, 0:gn], li_ps[:, 0:gn, D])
                    rd = r_p.tile([128, 7], F32, tag="rd", name="rd")
                    nc.vector.reciprocal(rd[:, 0:gn], da_ps[:, 0:gn, D])
                    nc.vector.tensor_tensor(
                        t1s[hh][half_ix][:, toff : toff + gn, :],
                        li_ps[:, 0:gn, 0:D],
                        rl[:, 0:gn].to_broadcast((128, gn, D)),
                        op=MUL,
                    )
                    nc.vector.tensor_tensor(
                        t2s[hh][half_ix][:, toff : toff + gn, :],
                        da_ps[:, 0:gn, 0:D],
                        rd[:, 0:gn].to_broadcast((128, gn, D)),
                        op=MUL,
                    )
                    if g == 2 or g == 5:  # half complete -> add + store
                        h = 2 * p + hh
                        eng = (
                            nc.vector
                            if (p == HPC // 2 - 1 and hh == 1)
                            else nc.gpsimd
                        )
                        eng.tensor_tensor(
                            outs[hh][half_ix][:],
                            t1s[hh][half_ix][:],
                            t2s[hh][half_ix][:],
                            op=ADD,
                        )
                        nc.sync.dma_start(
                            out_d[h, :, 16 * half_ix : 16 * half_ix + 16, :],
                            outs[hh][half_ix][:],
                        )

                # PE order: kv0, g0h0 (while head1 operands land), kv1, rest
                kv_chain(0)
                group(0, 0)
                kv_chain(1)
                group(0, 1)
                for g in range(1, len(GS)):
                    group(g, 0)
                    group(g, 1)
    nc.finalize()
    return nc


def _get_nc():
    if "nc" not in _cache:
        _cache["nc"] = _build()
    return _cache["nc"]


def _prep(q, k, v):
    q = np.asarray(q, dtype=np.float32).reshape(B * H, N, D)
    k = np.asarray(k, dtype=np.float32).reshape(B * H, N, D)
    v = np.asarray(v, dtype=np.float32).reshape(B * H, N, D)
    sq = float(np.std(q.astype(np.float64), ddof=1))
    sk = float(np.std(k.astype(np.float64), ddof=1))
    st = math.sqrt((sq * sq * sk * sk - B_CONST) / (2.0 * A_CONST))
    alpha = st / sq
    beta = st / sk

    # pair-packed d-major tensors [BH/2, 128, N]
    qt2 = np.ascontiguousarray(
        (alpha * q).reshape(B * H // 2, 2, N, D).transpose(0, 1, 3, 2)
    ).reshape(B * H // 2, 128, N).astype(_BF16)
    kt2 = np.ascontiguousarray(
        (k * (1.0 / (8.0 * alpha))).reshape(B * H // 2, 2, N, D).transpose(0, 1, 3, 2)
    ).reshape(B * H // 2, 128, N).astype(_BF16)
    # n-major partition-tiled exp(beta*k) and v_aug  [BH, 128, NT, D(+1)]
    ke = np.ascontiguousarray(
        np.exp(beta * k).reshape(B * H, NT, 128, D).transpose(0, 2, 1, 3)
    ).astype(_BF16)
    vaug = np.empty((B * H, N, D + 1), np.float32)
    vaug[:, :, 0:D] = v
    vaug[:, :, D] = 2.0
    va = np.ascontiguousarray(
        vaug.reshape(B * H, NT, 128, D + 1).transpose(0, 2, 1, 3)
    ).astype(_BF16)

    in_maps = []
    for c in range(N_CORES):
        hs = slice(c * HPC, (c + 1) * HPC)
        ps = slice(c * HPC // 2, (c + 1) * HPC // 2)
        in_maps.append(
            {
                "qt2": np.ascontiguousarray(qt2[ps]),
                "kt2": np.ascontiguousarray(kt2[ps]),
                "ke": np.ascontiguousarray(ke[hs]),
                "va": np.ascontiguousarray(va[hs]),
            }
        )
    return in_maps


def run_on_device(in_maps, **kw):
    from concourse.bass_utils import run_bass_kernel_spmd

    return run_bass_kernel_spmd(_get_nc(), in_maps, core_ids=list(range(N_CORES)), **kw)


def kernel(q, k, v):
    in_maps = _prep(q, k, v)
    res = run_on_device(in_maps)
    out = np.concatenate([r["out"] for r in res.results], axis=0)
    # [BH, 128, NT, D] bf16 -> [BH, N, D] fp32
    out = out.astype(np.float32).transpose(0, 2, 1, 3).reshape(B, H, N, D)
    return out


if __name__ == "__main__":
    nc = _get_nc()
    print("built ok")


# revision 7
# speedup vs baseline: 1.0738x; 1.0738x over previous
"""Trainium2 Bass kernel for LLN+diag attention (v5).

out = 0.5 * (lln_linear_attention(q,k,v) + block_diag_attention(q,k,v))

Shapes: q,k,v [4,16,4096,64] fp32.  8 NeuronCores, 8 heads per core.

Host prep (sharding/layout only; global std scalars are cross-device):
  qt2 [4,128,4096] bf16  pair-packed (alpha*q)^T  (exp -> lin Q; scores)
  kt2 [4,128,4096] bf16  pair-packed (k/(8 alpha))^T  (scores: qt*kt = q*k/8)
  ke  [8,128,32,64] bf16  exp(beta*k) n-major, PRE-EXPONENTIATED on host
  va  [8,128,32,65] bf16  v n-major + ones column of value 2.0
  out [8,128,32,64] bf16  device result; host transposes + upcasts to fp32

All DMA transfers are contiguous per-partition lines (no gather).  Pair 0's
qt2/kt2 loads are chunked 4x so its first score groups start early; later
pairs use whole-tile loads (better DMA packet efficiency) prefetched a full
pair ahead.  Per group the emission order is scores -> exp -> linear ->
diag so the activation latency hides behind the linear matmuls, with the
two heads' front/back halves interleaved.

The value-2.0 ones column doubles both paths' denominators, so adding the
two normalized halves yields the required 0.5*(lin+diag).

PSUM note: a matmul output chunk must not cross a 2KB PSUM bank line
(probed: crossing chunks corrupt), so group tiles are <= [128,7,65] = 1820B.
Group sizes [7,7,2 | 7,7,2] put a boundary at n-tile 16 so each half-head
output add/DMA fires as soon as its half is done.
"""

import math
import os
import sys

for _p in ("/opt/trn_rl_repo", "/opt/pypackages"):
    if os.path.isdir(_p) and _p not in sys.path:
        sys.path.insert(0, _p)

import numpy as np
import ml_dtypes

B, H, N, D = 4, 16, 4096, 64
N_CORES = 8
HPC = (B * H) // N_CORES          # heads per core = 8
NT = N // 128                     # 128-row n-tiles per head = 32
GS = [7, 7, 2, 7, 7, 2]           # group sizes; boundary at 16 for half adds
GOFF = [0, 7, 14, 16, 23, 30]
NCHUNK = 4                        # pair-0 qt2/kt2 column chunks of 1024
CW = N // NCHUNK
A_CONST = 0.14855178144710912
B_CONST = -0.35487039130661086

_BF16 = ml_dtypes.bfloat16

_cache = {}


def _build():
    import concourse.bass as bass
    import concourse.bacc as bacc
    import concourse.mybir as mybir
    from concourse.tile import TileContext

    dt = mybir.dt
    F32, BF = dt.float32, dt.bfloat16
    Exp = mybir.ActivationFunctionType.Exp
    Copy = mybir.ActivationFunctionType.Copy
    MUL = mybir.AluOpType.mult
    ADD = mybir.AluOpType.add

    nc = bacc.Bacc()
    qt2_d = nc.dram_tensor("qt2", [HPC // 2, 128, N], BF, kind="ExternalInput")
    kt2_d = nc.dram_tensor("kt2", [HPC // 2, 128, N], BF, kind="ExternalInput")
    ke_d = nc.dram_tensor("ke", [HPC, 128, NT, D], BF, kind="ExternalInput")
    va_d = nc.dram_tensor("va", [HPC, 128, NT, D + 1], BF, kind="ExternalInput")
    out_d = nc.dram_tensor("out", [HPC, 128, NT, D], BF, kind="ExternalOutput")

    with TileContext(nc) as tc:
        from contextlib import ExitStack

        with ExitStack() as ctx:
            chunk_p = ctx.enter_context(tc.tile_pool(name="chunk", bufs=1))
            pair_p = ctx.enter_context(tc.tile_pool(name="pair", bufs=2))
            head_p = ctx.enter_context(tc.tile_pool(name="head", bufs=3))
            kva_p = ctx.enter_context(tc.tile_pool(name="kva", bufs=2))
            at_p = ctx.enter_context(tc.tile_pool(name="attn", bufs=3))
            r_p = ctx.enter_context(tc.tile_pool(name="recip", bufs=4))
            t_p = ctx.enter_context(tc.tile_pool(name="tprod", bufs=2))
            o_p = ctx.enter_context(tc.tile_pool(name="outp", bufs=2))
            kv_ps_p = ctx.enter_context(tc.tile_pool(name="kvps", bufs=1, space="PSUM"))
            sc_ps_p = ctx.enter_context(tc.tile_pool(name="scps", bufs=2, space="PSUM"))
            da_ps_p = ctx.enter_context(tc.tile_pool(name="daps", bufs=3, space="PSUM"))
            li_ps_p = ctx.enter_context(tc.tile_pool(name="lips", bufs=2, space="PSUM"))

            for p in range(HPC // 2):  # head pairs; head 2p on parts 0:64, 2p+1 on 64:128
                chunked = p == 0
                # ---- DMAs: head0's kv operands first so PE starts ASAP ----
                ke0 = head_p.tile([128, NT, D], BF, tag="ke0")
                nc.sync.dma_start(ke0[:], ke_d[2 * p])
                va0 = head_p.tile([128, NT, D + 1], BF, tag="va0")
                nc.sync.dma_start(va0[:], va_d[2 * p])

                if chunked:
                    ktc, qtc, qec = [], [], []
                    for c in range(NCHUNK):
                        ktc.append(chunk_p.tile([128, CW], BF, tag=f"ktc{c}", name=f"ktc{c}"))
                        qtc.append(chunk_p.tile([128, CW], BF, tag=f"qtc{c}", name=f"qtc{c}"))
                        qec.append(chunk_p.tile([128, CW], BF, tag=f"qec{c}", name=f"qec{c}"))
                    for c in range(NCHUNK):
                        if c == 1:
                            ke1 = head_p.tile([128, NT, D], BF, tag="ke1")
                            nc.sync.dma_start(ke1[:], ke_d[2 * p + 1])
                            va1 = head_p.tile([128, NT, D + 1], BF, tag="va1")
                            nc.sync.dma_start(va1[:], va_d[2 * p + 1])
                        nc.sync.dma_start(ktc[c][:], kt2_d[p, :, c * CW : (c + 1) * CW])
                        nc.sync.dma_start(qtc[c][:], qt2_d[p, :, c * CW : (c + 1) * CW])
                        nc.scalar.activation(qec[c][:], qtc[c][:], Exp)

                    def kt_ap(rows, c0, w):  # absolute column c0, width w
                        c = c0 // CW
                        return ktc[c][rows, c0 - c * CW : c0 - c * CW + w]

                    def qt_ap(rows, c0, w):
                        c = c0 // CW
                        return qtc[c][rows, c0 - c * CW : c0 - c * CW + w]

                    def qe_ap(rows, c0, w):
                        c = c0 // CW
                        return qec[c][rows, c0 - c * CW : c0 - c * CW + w]
                else:
                    ktw = pair_p.tile([128, N], BF, tag="ktw")
                    nc.sync.dma_start(ktw[:], kt2_d[p])
                    qtw = pair_p.tile([128, N], BF, tag="qtw")
                    nc.sync.dma_start(qtw[:], qt2_d[p])
                    ke1 = head_p.tile([128, NT, D], BF, tag="ke1")
                    nc.sync.dma_start(ke1[:], ke_d[2 * p + 1])
                    va1 = head_p.tile([128, NT, D + 1], BF, tag="va1")
                    nc.sync.dma_start(va1[:], va_d[2 * p + 1])
                    qew = pair_p.tile([128, N], BF, tag="qew")
                    nc.scalar.activation(qew[:], qtw[:], Exp)

                    def kt_ap(rows, c0, w):
                        return ktw[rows, c0 : c0 + w]

                    def qt_ap(rows, c0, w):
                        return qtw[rows, c0 : c0 + w]

                    def qe_ap(rows, c0, w):
                        return qew[rows, c0 : c0 + w]

                kes, vas = [ke0, ke1], [va0, va1]
                t1s, t2s, outs = [], [], []
                for hh in range(2):
                    t1s.append([
                        t_p.tile([128, 16, D], BF, tag=f"t1h{hh}x{x}", name=f"t1h{hh}x{x}")
                        for x in range(2)
                    ])
                    t2s.append([
                        t_p.tile([128, 16, D], BF, tag=f"t2h{hh}x{x}", name=f"t2h{hh}x{x}")
                        for x in range(2)
                    ])
                    outs.append([
                        o_p.tile([128, 16, D], BF, tag=f"oh{hh}x{x}", name=f"oh{hh}x{x}")
                        for x in range(2)
                    ])

                kva = kva_p.tile([128, D + 1], BF, tag="kva")
                kv_ps = kv_ps_p.tile([128, D + 1], F32, tag="kv", name="kv")

                def kv_chain(hh):
                    ke, va = kes[hh], vas[hh]
                    for a in range(NT):
                        nc.tensor.matmul(
                            kv_ps[64 * hh : 64 * hh + 64, :],
                            lhsT=ke[:, a, :],
                            rhs=va[:, a, :],
                            start=(a == 0),
                            stop=(a == NT - 1),
                            tile_position=(0, 64 * hh),
                        )
                    nc.scalar.activation(
                        kva[64 * hh : 64 * hh + 64, :],
                        kv_ps[64 * hh : 64 * hh + 64, :],
                        Copy,
                    )

                live = {}

                def group_front(g, hh):
                    """scores + exp + linear numerators"""
                    goff, gn = GOFF[g], GS[g]
                    hp = 64 * hh
                    sc_ps = sc_ps_p.tile([128, 7, D], F32, tag="sc", name="sc")
                    for j in range(2 * gn):
                        i = j >> 1
                        half = j & 1
                        b = 2 * (goff + i) + half
                        nc.tensor.matmul(
                            sc_ps[64 * half : 64 * half + 64, i, :],
                            lhsT=kt_ap(slice(hp, hp + 64), 64 * b, 64),
                            rhs=qt_ap(slice(hp, hp + 64), 64 * b, 64),
                            start=True,
                            stop=True,
                            tile_position=(hp, 64 * half),
                        )
                    at_sb = at_p.tile([128, 7, D], BF, tag="at", name="at")
                    nc.scalar.activation(at_sb[:, 0:gn, :], sc_ps[:, 0:gn, :], Exp)
                    li_ps = li_ps_p.tile([128, 7, D + 1], F32, tag="li", name="li")
                    for i in range(gn):
                        a = goff + i
                        nc.tensor.matmul(
                            li_ps[:, i, :],
                            lhsT=qe_ap(slice(hp, hp + 64), 128 * a, 128),
                            rhs=kva[hp : hp + 64, :],
                            start=True,
                            stop=True,
                            tile_position=(hp, 0),
                        )
                    live[hh] = (at_sb, li_ps)

                def group_back(g, hh):
                    """diag numerators + normalize + (half) add/store"""
                    goff, gn = GOFF[g], GS[g]
                    half_ix = 0 if g < 3 else 1
                    toff = goff - 16 * half_ix
                    va = vas[hh]
                    at_sb, li_ps = live[hh]
                    da_ps = da_ps_p.tile([128, 7, D + 1], F32, tag="da", name="da")
                    for j in range(2 * gn):
                        i = j >> 1
                        half = j & 1
                        nc.tensor.matmul(
                            da_ps[64 * half : 64 * half + 64, i, :],
                            lhsT=at_sb[64 * half : 64 * half + 64, i, :],
                            rhs=va[64 * half : 64 * half + 64, goff + i, :],
                            start=True,
                            stop=True,
                            tile_position=(64 * half, 64 * half),
                        )
                    rl = r_p.tile([128, 7], F32, tag="rl", name="rl")
                    nc.vector.reciprocal(rl[:, 0:gn], li_ps[:, 0:gn, D])
                    rd = r_p.tile([128, 7], F32, tag="rd", name="rd")
                    nc.vector.reciprocal(rd[:, 0:gn], da_ps[:, 0:gn, D])
                    nc.vector.tensor_tensor(
                        t1s[hh][half_ix][:, toff : toff + gn, :],
                        li_ps[:, 0:gn, 0:D],
                        rl[:, 0:gn].to_broadcast((128, gn, D)),
                        op=MUL,
                    )
                    nc.vector.tensor_tensor(
                        t2s[hh][half_ix][:, toff : toff + gn, :],
                        da_ps[:, 0:gn, 0:D],
                        rd[:, 0:gn].to_broadcast((128, gn, D)),
                        op=MUL,
                    )
                    if g == 2 or g == 5:  # half complete -> add + store
                        h = 2 * p + hh
                        eng = nc.vector if p == HPC // 2 - 1 else nc.gpsimd
                        eng.tensor_tensor(
                            outs[hh][half_ix][:],
                            t1s[hh][half_ix][:],
                            t2s[hh][half_ix][:],
                            op=ADD,
                        )
                        nc.sync.dma_start(
                            out_d[h, :, 16 * half_ix : 16 * half_ix + 16, :],
                            outs[hh][half_ix][:],
                        )

                # PE order: kv0, g0h0-front (head1 operands still landing),
                # kv1, then fronts/backs interleaved to hide exp latency.
                kv_chain(0)
                group_front(0, 0)
                kv_chain(1)
                prev = (0, 0)
                for g in range(len(GS)):
                    for hh in range(2):
                        if (g, hh) == (0, 0):
                            continue
                        group_front(g, hh)
                        group_back(*prev)
                        prev = (g, hh)
                group_back(*prev)
    nc.finalize()
    return nc


def _get_nc():
    if "nc" not in _cache:
        _cache["nc"] = _build()
    return _cache["nc"]


def _prep(q, k, v):
    q = np.asarray(q, dtype=np.float32).reshape(B * H, N, D)
    k = np.asarray(k, dtype=np.float32).reshape(B * H, N, D)
    v = np.asarray(v, dtype=np.float32).reshape(B * H, N, D)
    sq = float(np.std(q.astype(np.float64), ddof=1))
    sk = float(np.std(k.astype(np.float64), ddof=1))
    st = math.sqrt((sq * sq * sk * sk - B_CONST) / (2.0 * A_CONST))
    alpha = st / sq
    beta = st / sk

    # pair-packed d-major tensors [BH/2, 128, N]
    qt2 = np.ascontiguousarray(
        (alpha * q).reshape(B * H // 2, 2, N, D).transpose(0, 1, 3, 2)
    ).reshape(B * H // 2, 128, N).astype(_BF16)
    kt2 = np.ascontiguousarray(
        (k * (1.0 / (8.0 * alpha))).reshape(B * H // 2, 2, N, D).transpose(0, 1, 3, 2)
    ).reshape(B * H // 2, 128, N).astype(_BF16)
    # n-major partition-tiled exp(beta*k) and v_aug  [BH, 128, NT, D(+1)]
    ke = np.ascontiguousarray(
        np.exp(beta * k).reshape(B * H, NT, 128, D).transpose(0, 2, 1, 3)
    ).astype(_BF16)
    vaug = np.empty((B * H, N, D + 1), np.float32)
    vaug[:, :, 0:D] = v
    vaug[:, :, D] = 2.0
    va = np.ascontiguousarray(
        vaug.reshape(B * H, NT, 128, D + 1).transpose(0, 2, 1, 3)
    ).astype(_BF16)

    in_maps = []
    for c in range(N_CORES):
        hs = slice(c * HPC, (c + 1) * HPC)
        ps = slice(c * HPC // 2, (c + 1) * HPC // 2)
        in_maps.append(
            {
                "qt2": np.ascontiguousarray(qt2[ps]),
                "kt2": np.ascontiguousarray(kt2[ps]),
                "ke": np.ascontiguousarray(ke[hs]),
                "va": np.ascontiguousarray(va[hs]),
            }
        )
    return in_maps


def run_on_device(in_maps, **kw):
    from concourse.bass_utils import run_bass_kernel_spmd

    return run_bass_kernel_spmd(_get_nc(), in_maps, core_ids=list(range(N_CORES)), **kw)


def kernel(q, k, v):
    in_maps = _prep(q, k, v)
    res = run_on_device(in_maps)
    out = np.concatenate([r["out"] for r in res.results], axis=0)
    # [BH, 128, NT, D] bf16 -> [BH, N, D] fp32
    out = out.astype(np.float32).transpose(0, 2, 1, 3).reshape(B, H, N, D)
    return out


if __name__ == "__main__":
    nc = _get_nc()
    print("built ok")


# revision 8
# speedup vs baseline: 1.0818x; 1.0075x over previous
"""Trainium2 Bass kernel for LLN+diag attention (v5).

out = 0.5 * (lln_linear_attention(q,k,v) + block_diag_attention(q,k,v))

Shapes: q,k,v [4,16,4096,64] fp32.  8 NeuronCores, 8 heads per core.

Host prep (sharding/layout only; global std scalars are cross-device):
  qt2 [4,128,4096] bf16  pair-packed (alpha*q)^T  (exp -> lin Q; scores)
  kt2 [4,128,4096] bf16  pair-packed (k/(8 alpha))^T  (scores: qt*kt = q*k/8)
  ke  [8,128,32,64] bf16  exp(beta*k) n-major, PRE-EXPONENTIATED on host
  va  [8,128,32,65] bf16  v n-major + ones column of value 2.0
  out [8,128,32,64] bf16  device result; host transposes + upcasts to fp32

All DMA transfers are contiguous per-partition lines (no gather).  Pair 0's
qt2/kt2 loads are chunked 4x so its first score groups start early; later
pairs use whole-tile loads (better DMA packet efficiency) prefetched a full
pair ahead.  Per group the emission order is scores -> exp -> linear ->
diag so the activation latency hides behind the linear matmuls, with the
two heads' front/back halves interleaved.

The value-2.0 ones column doubles both paths' denominators, so adding the
two normalized halves yields the required 0.5*(lin+diag).

PSUM note: a matmul output chunk must not cross a 2KB PSUM bank line
(probed: crossing chunks corrupt), so group tiles are <= [128,7,65] = 1820B.
Group sizes [7,7,2 | 7,7,2] put a boundary at n-tile 16 so each half-head
output add/DMA fires as soon as its half is done.
"""

import math
import os
import sys

for _p in ("/opt/trn_rl_repo", "/opt/pypackages"):
    if os.path.isdir(_p) and _p not in sys.path:
        sys.path.insert(0, _p)

import numpy as np
import ml_dtypes

B, H, N, D = 4, 16, 4096, 64
N_CORES = 8
HPC = (B * H) // N_CORES          # heads per core = 8
NT = N // 128                     # 128-row n-tiles per head = 32
GS = [7, 7, 2, 7, 7, 2]           # group sizes; boundary at 16 for half adds
GOFF = [0, 7, 14, 16, 23, 30]
NCHUNK = 4                        # pair-0 qt2/kt2 column chunks of 1024
CW = N // NCHUNK
A_CONST = 0.14855178144710912
B_CONST = -0.35487039130661086

_BF16 = ml_dtypes.bfloat16

_cache = {}


def _build():
    import concourse.bass as bass
    import concourse.bacc as bacc
    import concourse.mybir as mybir
    from concourse.tile import TileContext

    dt = mybir.dt
    F32, BF = dt.float32, dt.bfloat16
    Exp = mybir.ActivationFunctionType.Exp
    Copy = mybir.ActivationFunctionType.Copy
    MUL = mybir.AluOpType.mult
    ADD = mybir.AluOpType.add

    nc = bacc.Bacc()
    qt2_d = nc.dram_tensor("qt2", [HPC // 2, 128, N], BF, kind="ExternalInput")
    kt2_d = nc.dram_tensor("kt2", [HPC // 2, 128, N], BF, kind="ExternalInput")
    ke_d = nc.dram_tensor("ke", [HPC, 128, NT, D], BF, kind="ExternalInput")
    va_d = nc.dram_tensor("va", [HPC, 128, NT, D + 1], BF, kind="ExternalInput")
    out_d = nc.dram_tensor("out", [HPC, 128, NT, D], BF, kind="ExternalOutput")

    with TileContext(nc) as tc:
        from contextlib import ExitStack

        with ExitStack() as ctx:
            chunk_p = ctx.enter_context(tc.tile_pool(name="chunk", bufs=1))
            pair_p = ctx.enter_context(tc.tile_pool(name="pair", bufs=2))
            head_p = ctx.enter_context(tc.tile_pool(name="head", bufs=3))
            kva_p = ctx.enter_context(tc.tile_pool(name="kva", bufs=2))
            at_p = ctx.enter_context(tc.tile_pool(name="attn", bufs=3))
            r_p = ctx.enter_context(tc.tile_pool(name="recip", bufs=4))
            t_p = ctx.enter_context(tc.tile_pool(name="tprod", bufs=2))
            o_p = ctx.enter_context(tc.tile_pool(name="outp", bufs=2))
            kv_ps_p = ctx.enter_context(tc.tile_pool(name="kvps", bufs=1, space="PSUM"))
            sc_ps_p = ctx.enter_context(tc.tile_pool(name="scps", bufs=2, space="PSUM"))
            da_ps_p = ctx.enter_context(tc.tile_pool(name="daps", bufs=3, space="PSUM"))
            li_ps_p = ctx.enter_context(tc.tile_pool(name="lips", bufs=2, space="PSUM"))

            for p in range(HPC // 2):  # head pairs; head 2p on parts 0:64, 2p+1 on 64:128
                chunked = p == 0
                # ---- DMAs: head0's kv operands first so PE starts ASAP ----
                ke0 = head_p.tile([128, NT, D], BF, tag="ke0")
                nc.sync.dma_start(ke0[:], ke_d[2 * p])
                va0 = head_p.tile([128, NT, D + 1], BF, tag="va0")
                nc.sync.dma_start(va0[:], va_d[2 * p])

                if chunked:
                    ktc, qtc, qec = [], [], []
                    for c in range(NCHUNK):
                        ktc.append(chunk_p.tile([128, CW], BF, tag=f"ktc{c}", name=f"ktc{c}"))
                        qtc.append(chunk_p.tile([128, CW], BF, tag=f"qtc{c}", name=f"qtc{c}"))
                        qec.append(chunk_p.tile([128, CW], BF, tag=f"qec{c}", name=f"qec{c}"))
                    for c in range(NCHUNK):
                        if c == 1:
                            ke1 = head_p.tile([128, NT, D], BF, tag="ke1")
                            nc.sync.dma_start(ke1[:], ke_d[2 * p + 1])
                            va1 = head_p.tile([128, NT, D + 1], BF, tag="va1")
                            nc.sync.dma_start(va1[:], va_d[2 * p + 1])
                        nc.sync.dma_start(ktc[c][:], kt2_d[p, :, c * CW : (c + 1) * CW])
                        nc.sync.dma_start(qtc[c][:], qt2_d[p, :, c * CW : (c + 1) * CW])
                        nc.scalar.activation(qec[c][:], qtc[c][:], Exp)

                    def kt_ap(rows, c0, w):  # absolute column c0, width w
                        c = c0 // CW
                        return ktc[c][rows, c0 - c * CW : c0 - c * CW + w]

                    def qt_ap(rows, c0, w):
                        c = c0 // CW
                        return qtc[c][rows, c0 - c * CW : c0 - c * CW + w]

                    def qe_ap(rows, c0, w):
                        c = c0 // CW
                        return qec[c][rows, c0 - c * CW : c0 - c * CW + w]
                else:
                    ktw = pair_p.tile([128, N], BF, tag="ktw")
                    nc.sync.dma_start(ktw[:], kt2_d[p])
                    qtw = pair_p.tile([128, N], BF, tag="qtw")
                    nc.sync.dma_start(qtw[:], qt2_d[p])
                    ke1 = head_p.tile([128, NT, D], BF, tag="ke1")
                    nc.sync.dma_start(ke1[:], ke_d[2 * p + 1])
                    va1 = head_p.tile([128, NT, D + 1], BF, tag="va1")
                    nc.sync.dma_start(va1[:], va_d[2 * p + 1])
                    qew = pair_p.tile([128, N], BF, tag="qew")
                    nc.scalar.activation(qew[:], qtw[:], Exp)

                    def kt_ap(rows, c0, w):
                        return ktw[rows, c0 : c0 + w]

                    def qt_ap(rows, c0, w):
                        return qtw[rows, c0 : c0 + w]

                    def qe_ap(rows, c0, w):
                        return qew[rows, c0 : c0 + w]

                kes, vas = [ke0, ke1], [va0, va1]
                t1s, t2s, outs = [], [], []
                for hh in range(2):
                    t1s.append([
                        t_p.tile([128, 16, D], BF, tag=f"t1h{hh}x{x}", name=f"t1h{hh}x{x}")
                        for x in range(2)
                    ])
                    t2s.append([
                        t_p.tile([128, 16, D], BF, tag=f"t2h{hh}x{x}", name=f"t2h{hh}x{x}")
                        for x in range(2)
                    ])
                    outs.append([
                        o_p.tile([128, 16, D], BF, tag=f"oh{hh}x{x}", name=f"oh{hh}x{x}")
                        for x in range(2)
                    ])

                kva = kva_p.tile([128, D + 1], BF, tag="kva")
                kv_ps = kv_ps_p.tile([128, D + 1], F32, tag="kv", name="kv")

                def kv_chain(hh):
                    ke, va = kes[hh], vas[hh]
                    for a in range(NT):
                        nc.tensor.matmul(
                            kv_ps[64 * hh : 64 * hh + 64, :],
                            lhsT=ke[:, a, :],
                            rhs=va[:, a, :],
                            start=(a == 0),
                            stop=(a == NT - 1),
                            tile_position=(0, 64 * hh),
                        )
                    nc.scalar.activation(
                        kva[64 * hh : 64 * hh + 64, :],
                        kv_ps[64 * hh : 64 * hh + 64, :],
                        Copy,
                    )

                live = {}

                def group_front(g, hh):
                    """scores + exp + linear numerators"""
                    goff, gn = GOFF[g], GS[g]
                    hp = 64 * hh
                    sc_ps = sc_ps_p.tile([128, 7, D], F32, tag="sc", name="sc")
                    for j in range(2 * gn):
                        i = j >> 1
                        half = j & 1
                        b = 2 * (goff + i) + half
                        nc.tensor.matmul(
                            sc_ps[64 * half : 64 * half + 64, i, :],
                            lhsT=kt_ap(slice(hp, hp + 64), 64 * b, 64),
                            rhs=qt_ap(slice(hp, hp + 64), 64 * b, 64),
                            start=True,
                            stop=True,
                            tile_position=(hp, 64 * half),
                        )
                    at_sb = at_p.tile([128, 7, D], BF, tag="at", name="at")
                    nc.scalar.activation(at_sb[:, 0:gn, :], sc_ps[:, 0:gn, :], Exp)
                    li_ps = li_ps_p.tile([128, 7, D + 1], F32, tag="li", name="li")
                    for i in range(gn):
                        a = goff + i
                        nc.tensor.matmul(
                            li_ps[:, i, :],
                            lhsT=qe_ap(slice(hp, hp + 64), 128 * a, 128),
                            rhs=kva[hp : hp + 64, :],
                            start=True,
                            stop=True,
                            tile_position=(hp, 0),
                        )
                    live[hh] = (at_sb, li_ps)

                def group_back(g, hh):
                    """diag numerators + normalize + (half) add/store"""
                    goff, gn = GOFF[g], GS[g]
                    half_ix = 0 if g < 3 else 1
                    toff = goff - 16 * half_ix
                    va = vas[hh]
                    at_sb, li_ps = live[hh]
                    da_ps = da_ps_p.tile([128, 7, D + 1], F32, tag="da", name="da")
                    for j in range(2 * gn):
                        i = j >> 1
                        half = j & 1
                        nc.tensor.matmul(
                            da_ps[64 * half : 64 * half + 64, i, :],
                            lhsT=at_sb[64 * half : 64 * half + 64, i, :],
                            rhs=va[64 * half : 64 * half + 64, goff + i, :],
                            start=True,
                            stop=True,
                            tile_position=(64 * half, 64 * half),
                        )
                    rl = r_p.tile([128, 7], F32, tag="rl", name="rl")
                    nc.vector.reciprocal(rl[:, 0:gn], li_ps[:, 0:gn, D])
                    rd = r_p.tile([128, 7], F32, tag="rd", name="rd")
                    nc.vector.reciprocal(rd[:, 0:gn], da_ps[:, 0:gn, D])
                    nc.vector.tensor_tensor(
                        t1s[hh][half_ix][:, toff : toff + gn, :],
                        li_ps[:, 0:gn, 0:D],
                        rl[:, 0:gn].to_broadcast((128, gn, D)),
                        op=MUL,
                    )
                    nc.vector.tensor_tensor(
                        t2s[hh][half_ix][:, toff : toff + gn, :],
                        da_ps[:, 0:gn, 0:D],
                        rd[:, 0:gn].to_broadcast((128, gn, D)),
                        op=MUL,
                    )
                    if g == 2 or g == 5:  # half complete -> add + store
                        h = 2 * p + hh
                        eng = nc.vector if p == HPC // 2 - 1 else nc.gpsimd
                        eng.tensor_tensor(
                            outs[hh][half_ix][:],
                            t1s[hh][half_ix][:],
                            t2s[hh][half_ix][:],
                            op=ADD,
                        )
                        # separate HWDGE queue: keeps compute-gated output
                        # stores from head-of-line-blocking input loads
                        nc.scalar.dma_start(
                            out_d[h, :, 16 * half_ix : 16 * half_ix + 16, :],
                            outs[hh][half_ix][:],
                        )

                # PE order: kv0, g0h0-front (head1 operands still landing),
                # kv1, then fronts/backs interleaved to hide exp latency.
                kv_chain(0)
                group_front(0, 0)
                kv_chain(1)
                prev = (0, 0)
                for g in range(len(GS)):
                    for hh in range(2):
                        if (g, hh) == (0, 0):
                            continue
                        group_front(g, hh)
                        group_back(*prev)
                        prev = (g, hh)
                group_back(*prev)
    nc.finalize()
    return nc


def _get_nc():
    if "nc" not in _cache:
        _cache["nc"] = _build()
    return _cache["nc"]


def _prep(q, k, v):
    q = np.asarray(q, dtype=np.float32).reshape(B * H, N, D)
    k = np.asarray(k, dtype=np.float32).reshape(B * H, N, D)
    v = np.asarray(v, dtype=np.float32).reshape(B * H, N, D)
    sq = float(np.std(q.astype(np.float64), ddof=1))
    sk = float(np.std(k.astype(np.float64), ddof=1))
    st = math.sqrt((sq * sq * sk * sk - B_CONST) / (2.0 * A_CONST))
    alpha = st / sq
    beta = st / sk

    # pair-packed d-major tensors [BH/2, 128, N]
    qt2 = np.ascontiguousarray(
        (alpha * q).reshape(B * H // 2, 2, N, D).transpose(0, 1, 3, 2)
    ).reshape(B * H // 2, 128, N).astype(_BF16)
    kt2 = np.ascontiguousarray(
        (k * (1.0 / (8.0 * alpha))).reshape(B * H // 2, 2, N, D).transpose(0, 1, 3, 2)
    ).reshape(B * H // 2, 128, N).astype(_BF16)
    # n-major partition-tiled exp(beta*k) and v_aug  [BH, 128, NT, D(+1)]
    ke = np.ascontiguousarray(
        np.exp(beta * k).reshape(B * H, NT, 128, D).transpose(0, 2, 1, 3)
    ).astype(_BF16)
    vaug = np.empty((B * H, N, D + 1), np.float32)
    vaug[:, :, 0:D] = v
    vaug[:, :, D] = 2.0
    va = np.ascontiguousarray(
        vaug.reshape(B * H, NT, 128, D + 1).transpose(0, 2, 1, 3)
    ).astype(_BF16)

    in_maps = []
    for c in range(N_CORES):
        hs = slice(c * HPC, (c + 1) * HPC)
        ps = slice(c * HPC // 2, (c + 1) * HPC // 2)
        in_maps.append(
            {
                "qt2": np.ascontiguousarray(qt2[ps]),
                "kt2": np.ascontiguousarray(kt2[ps]),
                "ke": np.ascontiguousarray(ke[hs]),
                "va": np.ascontiguousarray(va[hs]),
            }
        )
    return in_maps


def run_on_device(in_maps, **kw):
    from concourse.bass_utils import run_bass_kernel_spmd

    return run_bass_kernel_spmd(_get_nc(), in_maps, core_ids=list(range(N_CORES)), **kw)


def kernel(q, k, v):
    in_maps = _prep(q, k, v)
    res = run_on_device(in_maps)
    out = np.concatenate([r["out"] for r in res.results], axis=0)
    # [BH, 128, NT, D] bf16 -> [BH, N, D] fp32
    out = out.astype(np.float32).transpose(0, 2, 1, 3).reshape(B, H, N, D)
    return out


if __name__ == "__main__":
    nc = _get_nc()
    print("built ok")


# revision 9
# speedup vs baseline: 1.1016x; 1.0182x over previous
"""Trainium2 Bass kernel for LLN+diag attention (v5).

out = 0.5 * (lln_linear_attention(q,k,v) + block_diag_attention(q,k,v))

Shapes: q,k,v [4,16,4096,64] fp32.  8 NeuronCores, 8 heads per core.

Host prep (sharding/layout only; global std scalars are cross-device):
  qt2 [4,128,4096] bf16  pair-packed (alpha*q)^T  (exp -> lin Q; scores)
  kt2 [4,128,4096] bf16  pair-packed (k/(8 alpha))^T  (scores: qt*kt = q*k/8)
  ke  [8,128,32,64] bf16  exp(beta*k) n-major, PRE-EXPONENTIATED on host
  va  [8,128,32,65] bf16  v n-major + ones column of value 2.0
  out [8,128,32,64] bf16  device result; host transposes + upcasts to fp32

All DMA transfers are contiguous per-partition lines (no gather).  Pair 0's
qt2/kt2 loads are chunked 4x so its first score groups start early; later
pairs use whole-tile loads (better DMA packet efficiency) prefetched a full
pair ahead.  Per group the emission order is scores -> exp -> linear ->
diag so the activation latency hides behind the linear matmuls, with the
two heads' front/back halves interleaved.

The value-2.0 ones column doubles both paths' denominators, so adding the
two normalized halves yields the required 0.5*(lin+diag).

PSUM note: a matmul output chunk must not cross a 2KB PSUM bank line
(probed: crossing chunks corrupt), so group tiles are <= [128,7,65] = 1820B.
Group sizes [7,7,2 | 7,7,2] put a boundary at n-tile 16 so each half-head
output add/DMA fires as soon as its half is done.
"""

import math
import os
import sys

for _p in ("/opt/trn_rl_repo", "/opt/pypackages"):
    if os.path.isdir(_p) and _p not in sys.path:
        sys.path.insert(0, _p)

import numpy as np
import ml_dtypes

B, H, N, D = 4, 16, 4096, 64
N_CORES = 8
HPC = (B * H) // N_CORES          # heads per core = 8
NT = N // 128                     # 128-row n-tiles per head = 32
GS = [7, 7, 2, 7, 7, 2]           # group sizes; boundary at 16 for half adds
GOFF = [0, 7, 14, 16, 23, 30]
NCHUNK = 4                        # pair-0 qt2/kt2 column chunks of 1024
CW = N // NCHUNK
A_CONST = 0.14855178144710912
B_CONST = -0.35487039130661086

_BF16 = ml_dtypes.bfloat16

_cache = {}


def _build():
    import concourse.bass as bass
    import concourse.bacc as bacc
    import concourse.mybir as mybir
    from concourse.tile import TileContext

    dt = mybir.dt
    F32, BF = dt.float32, dt.bfloat16
    Exp = mybir.ActivationFunctionType.Exp
    Copy = mybir.ActivationFunctionType.Copy
    MUL = mybir.AluOpType.mult
    ADD = mybir.AluOpType.add

    nc = bacc.Bacc()
    qt2_d = nc.dram_tensor("qt2", [HPC // 2, 128, N], BF, kind="ExternalInput")
    kt2_d = nc.dram_tensor("kt2", [HPC // 2, 128, N], BF, kind="ExternalInput")
    ke_d = nc.dram_tensor("ke", [HPC, 128, NT, D], BF, kind="ExternalInput")
    va_d = nc.dram_tensor("va", [HPC, 128, NT, D + 1], BF, kind="ExternalInput")
    out_d = nc.dram_tensor("out", [HPC, 128, NT, D], BF, kind="ExternalOutput")

    with TileContext(nc) as tc:
        from contextlib import ExitStack

        with ExitStack() as ctx:
            chunk_p = ctx.enter_context(tc.tile_pool(name="chunk", bufs=1))
            pairin_p = ctx.enter_context(tc.tile_pool(name="pairin", bufs=3))
            qew_p = ctx.enter_context(tc.tile_pool(name="qew", bufs=2))
            head_p = ctx.enter_context(tc.tile_pool(name="head", bufs=3))
            kva_p = ctx.enter_context(tc.tile_pool(name="kva", bufs=2))
            at_p = ctx.enter_context(tc.tile_pool(name="attn", bufs=3))
            r_p = ctx.enter_context(tc.tile_pool(name="recip", bufs=4))
            t_p = ctx.enter_context(tc.tile_pool(name="tprod", bufs=2))
            o_p = ctx.enter_context(tc.tile_pool(name="outp", bufs=2))
            kv_ps_p = ctx.enter_context(tc.tile_pool(name="kvps", bufs=1, space="PSUM"))
            sc_ps_p = ctx.enter_context(tc.tile_pool(name="scps", bufs=2, space="PSUM"))
            da_ps_p = ctx.enter_context(tc.tile_pool(name="daps", bufs=3, space="PSUM"))
            li_ps_p = ctx.enter_context(tc.tile_pool(name="lips", bufs=2, space="PSUM"))

            for p in range(HPC // 2):  # head pairs; head 2p on parts 0:64, 2p+1 on 64:128
                chunked = p == 0
                # ---- DMAs: head0's kv operands first so PE starts ASAP ----
                ke0 = head_p.tile([128, NT, D], BF, tag="ke0")
                va0 = head_p.tile([128, NT, D + 1], BF, tag="va0")
                if p == 0:
                    nc.sync.dma_start(ke0[:, 0:16, :], ke_d[2 * p, :, 0:16, :])
                    nc.sync.dma_start(va0[:, 0:16, :], va_d[2 * p, :, 0:16, :])
                    nc.sync.dma_start(ke0[:, 16:NT, :], ke_d[2 * p, :, 16:NT, :])
                    nc.sync.dma_start(va0[:, 16:NT, :], va_d[2 * p, :, 16:NT, :])
                else:
                    nc.sync.dma_start(ke0[:], ke_d[2 * p])
                    nc.sync.dma_start(va0[:], va_d[2 * p])

                if chunked:
                    ktc, qtc, qec = [], [], []
                    for c in range(NCHUNK):
                        ktc.append(chunk_p.tile([128, CW], BF, tag=f"ktc{c}", name=f"ktc{c}"))
                        qtc.append(chunk_p.tile([128, CW], BF, tag=f"qtc{c}", name=f"qtc{c}"))
                        qec.append(chunk_p.tile([128, CW], BF, tag=f"qec{c}", name=f"qec{c}"))
                    for c in range(NCHUNK):
                        if c == 1:
                            ke1 = head_p.tile([128, NT, D], BF, tag="ke1")
                            nc.sync.dma_start(ke1[:], ke_d[2 * p + 1])
                            va1 = head_p.tile([128, NT, D + 1], BF, tag="va1")
                            nc.sync.dma_start(va1[:], va_d[2 * p + 1])
                        nc.sync.dma_start(ktc[c][:], kt2_d[p, :, c * CW : (c + 1) * CW])
                        nc.sync.dma_start(qtc[c][:], qt2_d[p, :, c * CW : (c + 1) * CW])
                        nc.scalar.activation(qec[c][:], qtc[c][:], Exp)

                    def kt_ap(rows, c0, w):  # absolute column c0, width w
                        c = c0 // CW
                        return ktc[c][rows, c0 - c * CW : c0 - c * CW + w]

                    def qt_ap(rows, c0, w):
                        c = c0 // CW
                        return qtc[c][rows, c0 - c * CW : c0 - c * CW + w]

                    def qe_ap(rows, c0, w):
                        c = c0 // CW
                        return qec[c][rows, c0 - c * CW : c0 - c * CW + w]
                else:
                    ktw = pairin_p.tile([128, N], BF, tag="ktw")
                    nc.sync.dma_start(ktw[:], kt2_d[p])
                    qtw = pairin_p.tile([128, N], BF, tag="qtw")
                    nc.sync.dma_start(qtw[:], qt2_d[p])
                    ke1 = head_p.tile([128, NT, D], BF, tag="ke1")
                    nc.sync.dma_start(ke1[:], ke_d[2 * p + 1])
                    va1 = head_p.tile([128, NT, D + 1], BF, tag="va1")
                    nc.sync.dma_start(va1[:], va_d[2 * p + 1])
                    qew = qew_p.tile([128, N], BF, tag="qew")
                    nc.scalar.activation(qew[:], qtw[:], Exp)

                    def kt_ap(rows, c0, w):
                        return ktw[rows, c0 : c0 + w]

                    def qt_ap(rows, c0, w):
                        return qtw[rows, c0 : c0 + w]

                    def qe_ap(rows, c0, w):
                        return qew[rows, c0 : c0 + w]

                kes, vas = [ke0, ke1], [va0, va1]
                t1s, t2s, outs = [], [], []
                for hh in range(2):
                    t1s.append([
                        t_p.tile([128, 16, D], BF, tag=f"t1h{hh}x{x}", name=f"t1h{hh}x{x}")
                        for x in range(2)
                    ])
                    t2s.append([
                        t_p.tile([128, 16, D], BF, tag=f"t2h{hh}x{x}", name=f"t2h{hh}x{x}")
                        for x in range(2)
                    ])
                    outs.append([
                        o_p.tile([128, 16, D], BF, tag=f"oh{hh}x{x}", name=f"oh{hh}x{x}")
                        for x in range(2)
                    ])

                kva = kva_p.tile([128, D + 1], BF, tag="kva")
                kv_ps = kv_ps_p.tile([128, D + 1], F32, tag="kv", name="kv")

                def kv_chain(hh):
                    ke, va = kes[hh], vas[hh]
                    for a in range(NT):
                        nc.tensor.matmul(
                            kv_ps[64 * hh : 64 * hh + 64, :],
                            lhsT=ke[:, a, :],
                            rhs=va[:, a, :],
                            start=(a == 0),
                            stop=(a == NT - 1),
                            tile_position=(0, 64 * hh),
                        )
                    nc.scalar.activation(
                        kva[64 * hh : 64 * hh + 64, :],
                        kv_ps[64 * hh : 64 * hh + 64, :],
                        Copy,
                    )

                live = {}

                def group_front(g, hh):
                    """scores + exp + linear numerators"""
                    goff, gn = GOFF[g], GS[g]
                    hp = 64 * hh
                    sc_ps = sc_ps_p.tile([128, 7, D], F32, tag="sc", name="sc")
                    for j in range(2 * gn):
                        i = j >> 1
                        half = j & 1
                        b = 2 * (goff + i) + half
                        nc.tensor.matmul(
                            sc_ps[64 * half : 64 * half + 64, i, :],
                            lhsT=kt_ap(slice(hp, hp + 64), 64 * b, 64),
                            rhs=qt_ap(slice(hp, hp + 64), 64 * b, 64),
                            start=True,
                            stop=True,
                            tile_position=(hp, 64 * half),
                        )
                    at_sb = at_p.tile([128, 7, D], BF, tag="at", name="at")
                    nc.scalar.activation(at_sb[:, 0:gn, :], sc_ps[:, 0:gn, :], Exp)
                    li_ps = li_ps_p.tile([128, 7, D + 1], F32, tag="li", name="li")
                    for i in range(gn):
                        a = goff + i
                        nc.tensor.matmul(
                            li_ps[:, i, :],
                            lhsT=qe_ap(slice(hp, hp + 64), 128 * a, 128),
                            rhs=kva[hp : hp + 64, :],
                            start=True,
                            stop=True,
                            tile_position=(hp, 0),
                        )
                    live[hh] = (at_sb, li_ps)

                def group_back(g, hh):
                    """diag numerators + normalize + (half) add/store"""
                    goff, gn = GOFF[g], GS[g]
                    half_ix = 0 if g < 3 else 1
                    toff = goff - 16 * half_ix
                    va = vas[hh]
                    at_sb, li_ps = live[hh]
                    da_ps = da_ps_p.tile([128, 7, D + 1], F32, tag="da", name="da")
                    for j in range(2 * gn):
                        i = j >> 1
                        half = j & 1
                        nc.tensor.matmul(
                            da_ps[64 * half : 64 * half + 64, i, :],
                            lhsT=at_sb[64 * half : 64 * half + 64, i, :],
                            rhs=va[64 * half : 64 * half + 64, goff + i, :],
                            start=True,
                            stop=True,
                            tile_position=(64 * half, 64 * half),
                        )
                    rl = r_p.tile([128, 7], F32, tag="rl", name="rl")
                    nc.vector.reciprocal(rl[:, 0:gn], li_ps[:, 0:gn, D])
                    rd = r_p.tile([128, 7], F32, tag="rd", name="rd")
                    nc.vector.reciprocal(rd[:, 0:gn], da_ps[:, 0:gn, D])
                    nc.vector.tensor_tensor(
                        t1s[hh][half_ix][:, toff : toff + gn, :],
                        li_ps[:, 0:gn, 0:D],
                        rl[:, 0:gn].to_broadcast((128, gn, D)),
                        op=MUL,
                    )
                    nc.vector.tensor_tensor(
                        t2s[hh][half_ix][:, toff : toff + gn, :],
                        da_ps[:, 0:gn, 0:D],
                        rd[:, 0:gn].to_broadcast((128, gn, D)),
                        op=MUL,
                    )
                    if g == 2 or g == 5:  # half complete -> add + store
                        h = 2 * p + hh
                        eng = nc.vector if p == HPC // 2 - 1 else nc.gpsimd
                        eng.tensor_tensor(
                            outs[hh][half_ix][:],
                            t1s[hh][half_ix][:],
                            t2s[hh][half_ix][:],
                            op=ADD,
                        )
                        # separate HWDGE queue: keeps compute-gated output
                        # stores from head-of-line-blocking input loads
                        nc.scalar.dma_start(
                            out_d[h, :, 16 * half_ix : 16 * half_ix + 16, :],
                            outs[hh][half_ix][:],
                        )

                # PE order: kv0, g0h0-front (head1 operands still landing),
                # kv1, then fronts/backs interleaved to hide exp latency.
                kv_chain(0)
                group_front(0, 0)
                kv_chain(1)
                prev = (0, 0)
                for g in range(len(GS)):
                    for hh in range(2):
                        if (g, hh) == (0, 0):
                            continue
                        group_front(g, hh)
                        group_back(*prev)
                        prev = (g, hh)
                group_back(*prev)
    nc.finalize()
    return nc


def _get_nc():
    if "nc" not in _cache:
        _cache["nc"] = _build()
    return _cache["nc"]


def _prep(q, k, v):
    q = np.asarray(q, dtype=np.float32).reshape(B * H, N, D)
    k = np.asarray(k, dtype=np.float32).reshape(B * H, N, D)
    v = np.asarray(v, dtype=np.float32).reshape(B * H, N, D)
    sq = float(np.std(q.astype(np.float64), ddof=1))
    sk = float(np.std(k.astype(np.float64), ddof=1))
    st = math.sqrt((sq * sq * sk * sk - B_CONST) / (2.0 * A_CONST))
    alpha = st / sq
    beta = st / sk

    # pair-packed d-major tensors [BH/2, 128, N]
    qt2 = np.ascontiguousarray(
        (alpha * q).reshape(B * H // 2, 2, N, D).transpose(0, 1, 3, 2)
    ).reshape(B * H // 2, 128, N).astype(_BF16)
    kt2 = np.ascontiguousarray(
        (k * (1.0 / (8.0 * alpha))).reshape(B * H // 2, 2, N, D).transpose(0, 1, 3, 2)
    ).reshape(B * H // 2, 128, N).astype(_BF16)
    # n-major partition-tiled exp(beta*k) and v_aug  [BH, 128, NT, D(+1)]
    ke = np.ascontiguousarray(
        np.exp(beta * k).reshape(B * H, NT, 128, D).transpose(0, 2, 1, 3)
    ).astype(_BF16)
    vaug = np.empty((B * H, N, D + 1), np.float32)
    vaug[:, :, 0:D] = v
    vaug[:, :, D] = 2.0
    va = np.ascontiguousarray(
        vaug.reshape(B * H, NT, 128, D + 1).transpose(0, 2, 1, 3)
    ).astype(_BF16)

    in_maps = []
    for c in range(N_CORES):
        hs = slice(c * HPC, (c + 1) * HPC)
        ps = slice(c * HPC // 2, (c + 1) * HPC // 2)
        in_maps.append(
            {
                "qt2": np.ascontiguousarray(qt2[ps]),
                "kt2": np.ascontiguousarray(kt2[ps]),
                "ke": np.ascontiguousarray(ke[hs]),
                "va": np.ascontiguousarray(va[hs]),
            }
        )
    return in_maps


def run_on_device(in_maps, **kw):
    from concourse.bass_utils import run_bass_kernel_spmd

    return run_bass_kernel_spmd(_get_nc(), in_maps, core_ids=list(range(N_CORES)), **kw)


def kernel(q, k, v):
    in_maps = _prep(q, k, v)
    res = run_on_device(in_maps)
    out = np.concatenate([r["out"] for r in res.results], axis=0)
    # [BH, 128, NT, D] bf16 -> [BH, N, D] fp32
    out = out.astype(np.float32).transpose(0, 2, 1, 3).reshape(B, H, N, D)
    return out


if __name__ == "__main__":
    nc = _get_nc()
    print("built ok")
